# revision 8
# baseline (speedup 1.0000x reference)
"""Triplet-margin loss (EuclideanTriple) on 8 Trainium2 NeuronCores.

loss = sum_i relu( ||x_i - y_i + eps||_2 + margin - ||x_i - z_i + eps||_2 )

Data-parallel: N=131072 rows sharded 8 ways (16384 rows/core, no
collectives). Each core reduces its hinge terms to per-partition sums
([128,4]); the host sums the 8 partials into the final scalar.

Per-core layout: rows -> partitions. The shard is processed as 15 "main"
chunks of 8 rows/partition (1 MiB triple-DMAs, quadruple buffered) plus a
tapered tail of 4,2,1,1-row chunks. The kernel is HBM-read bound
(48 MiB/core at ~360 GB/s -> ~140 us); the taper exists to shrink the
*drain* — the serial chain between the last DMA landing and the final
hinge value — from ~16 us (uniform chunks + monolithic tail) to ~4 us:

  - main chunks: DVE subs, ACT Square(+eps bias) with per-row accum_out
    (rows 0..3) / bulk Square + DVE reduce (rows 4..7), as before.
  - tail A (after the last main chunk, overlapped with the taper DMAs):
    ACT Sqrt on both accumulators, DVE hinge sub, DVE
    tensor_scalar((d+margin) max 0) with accum_out -> per-partition sums.
  - taper chunks: all-DVE (sub + per-row tensor_tensor_reduce square+sum),
    so the ACT activation table stays on Sqrt.
  - tail B (the drain): ACT Sqrt on the 16 taper columns (no table
    reload), DVE hinge + fused relu/accum, out DMA on the SP ring.

The out DMA must stay on nc.sync: measured, moving it to the ACT HWDGE
ring or gpsimd SWDGE costs +20 us/pass, and SWDGE bulk loads cost
+31 us/pass (sweep 2026-08-08). eps is folded into the ACT Square bias
on main chunks and dropped on taper rows (|relative error| ~1e-7, far
below f32 rounding at these magnitudes).
"""

from contextlib import ExitStack

import numpy as np

import concourse.bacc as bacc
import concourse.bass as bass
import concourse.mybir as mybir
import concourse.tile as tile
from concourse import bass_utils

N_TOTAL = 131072
D = 256
N_CORES = 8
SHARD = N_TOTAL // N_CORES  # 16384 rows per core
P = 128                     # SBUF partitions
RPP = SHARD // P            # 128 rows per partition (whole shard)
CHUNK_A = 8                 # rows per partition per main chunk (1 MiB DMAs)
TAPER = (4, 2, 1, 1)        # trailing chunk sizes (rows per partition)
MARGIN = 0.5
EPS = 1e-6
F32 = mybir.dt.float32
IO_BUFS = 4
ACT_ROWS = 4  # rows per tensor per main chunk squared+reduced on ACT
OUT_COLS = 4


def build_nc(
    repeat: int = 1,
    mode: str = "full",
    act_rows: int = ACT_ROWS,
    io_bufs: int = IO_BUFS,
    loop: bool = False,
    chunk_a: int = CHUNK_A,
    taper: tuple = TAPER,
    load_rings: str = "sp",     # 'sp' | 'sp_act'
) -> bass.Bass:
    """mode: 'full' | 'dma' (loads only) | 'compute' (no input loads).
    loop=True wraps the repeats in a For_i hardware loop (for timing runs
    with large repeat counts without unrolled instruction blowup)."""

    nt = sum(taper)
    n_main = (RPP - nt) // chunk_a
    assert n_main * chunk_a + nt == RPP
    fd = chunk_a * D
    nc = bacc.Bacc("TRN2", target_bir_lowering=False, debug=False)
    x = nc.dram_tensor("x", [SHARD, D], F32, kind="ExternalInput").ap()
    y = nc.dram_tensor("y", [SHARD, D], F32, kind="ExternalInput").ap()
    z = nc.dram_tensor("z", [SHARD, D], F32, kind="ExternalInput").ap()
    # per-partition partial hinge sums: col0 ACT-rows path, col1 DVE-rows
    # path, col2 taper path, col3 always zero
    out = nc.dram_tensor("out", [P, OUT_COLS], F32, kind="ExternalOutput").ap()

    act = mybir.ActivationFunctionType
    alu = mybir.AluOpType

    with tile.TileContext(nc) as tc:
        with ExitStack() as ctx:
            io = ctx.enter_context(tc.tile_pool(name="io", bufs=io_bufs))
            tio = ctx.enter_context(tc.tile_pool(name="tio", bufs=1))
            acc = ctx.enter_context(tc.tile_pool(name="acc", bufs=1))

            # Per-row squared distances, one accumulator per writing engine
            # (a shared tile would WAW-serialize ACT vs DVE):
            #   dsq_act: ACT accum_out rows   (act_rows per main chunk)
            #   dsq_dve: DVE tensor_reduce rows (rest of each main chunk)
            #   dsq_tap: DVE tensor_tensor_reduce rows (taper chunks)
            # Each is [pos | neg] halves with matching row order.
            dve_rows = chunk_a - act_rows
            na = n_main * act_rows
            nd = n_main * dve_rows
            dsq_act = acc.tile([P, max(2 * na, 1)], F32, tag="dsq_act")
            dsq_dve = acc.tile([P, max(2 * nd, 1)], F32, tag="dsq_dve")
            dsq_tap = acc.tile([P, max(2 * nt, 1)], F32, tag="dsq_tap")
            hsum = acc.tile([P, OUT_COLS], F32, tag="hsum")
            nc.vector.memset(hsum[:], 0.0)

            # const bias vector for ACT (bias must be an AP)
            eps_t = acc.tile([P, 1], F32, tag="eps")
            nc.vector.memset(eps_t[:], EPS)

            if mode == "compute":
                for _ in range(io_bufs):
                    for tag in ("xt", "yt", "zt"):
                        t = io.tile([P, fd], F32, tag=tag)
                        nc.vector.memset(t[:], 0.0)

            def load(xt, yt, zt, rows):
                y_eng = nc.scalar if load_rings == "sp_act" else nc.sync
                nc.sync.dma_start(
                    xt[:], x[rows, :].rearrange("(p a) d -> p (a d)", p=P)
                )
                y_eng.dma_start(
                    yt[:], y[rows, :].rearrange("(p a) d -> p (a d)", p=P)
                )
                nc.sync.dma_start(
                    zt[:], z[rows, :].rearrange("(p a) d -> p (a d)", p=P)
                )

            def hinge(dsq_t, n_cols, out_col, tag):
                """sqrt both halves, then per-partition sum of
                max(0, dpos + margin - dneg) into hsum[:, out_col]."""
                nc.scalar.activation(dsq_t[:], dsq_t[:], act.Sqrt)
                hing = acc.tile([P, n_cols], F32, tag=f"hing{tag}")
                nc.vector.tensor_sub(
                    hing[:], dsq_t[:, :n_cols], dsq_t[:, n_cols:]
                )
                relu_t = acc.tile([P, n_cols], F32, tag=f"relu{tag}")
                # out = (hing + margin) max 0; with accum_out, op1 would
                # become the reduce op instead, so the row-sum is a second
                # tensor_scalar whose op1=add reduces into hsum.
                nc.vector.tensor_scalar(
                    relu_t[:], hing[:], MARGIN, 0.0, alu.add, alu.max
                )
                nc.vector.tensor_scalar(
                    hing[:],
                    relu_t[:],
                    0.0,
                    None,
                    alu.add,
                    alu.add,
                    accum_out=hsum[:, out_col : out_col + 1],
                )

            def rep_body():
                for c in range(n_main):
                    rows = slice(c * P * chunk_a, (c + 1) * P * chunk_a)
                    xt = io.tile([P, fd], F32, tag="xt")
                    yt = io.tile([P, fd], F32, tag="yt")
                    zt = io.tile([P, fd], F32, tag="zt")
                    if mode != "compute":
                        load(xt, yt, zt, rows)
                    if mode == "dma":
                        continue
                    # u = x - y in place into the y/z tiles, then (u + eps)^2
                    # on ACT (the +eps rides ACT's free bias). Per-row
                    # square+reduce is split: the first act_rows rows of each
                    # tile go through per-row ACT calls whose accum_out
                    # directly yields the row's sum; the remaining rows get
                    # one bulk ACT square + a DVE tensor_reduce.
                    nc.vector.tensor_sub(yt[:], xt[:], yt[:])
                    nc.vector.tensor_sub(zt[:], xt[:], zt[:])
                    for half, t in ((0, yt), (1, zt)):
                        for r in range(act_rows):
                            col = half * na + c * act_rows + r
                            nc.scalar.activation(
                                t[:, r * D : (r + 1) * D],
                                t[:, r * D : (r + 1) * D],
                                act.Square,
                                bias=eps_t[:],
                                accum_out=dsq_act[:, col : col + 1],
                            )
                        if dve_rows:
                            base = half * nd + c * dve_rows
                            nc.scalar.activation(
                                t[:, act_rows * D :],
                                t[:, act_rows * D :],
                                act.Square,
                                bias=eps_t[:],
                            )
                            nc.vector.reduce_sum(
                                dsq_dve[:, base : base + dve_rows],
                                t[:, act_rows * D :].rearrange(
                                    "p (a d) -> p a d", a=dve_rows
                                ),
                                axis=mybir.AxisListType.X,
                            )

                if mode == "full":
                    # tail A: hinge all main-chunk rows now, overlapped with
                    # the taper chunks' DMAs, so the post-last-DMA drain only
                    # covers the final 1-row chunk.
                    hinge(dsq_act, na, 0, "a")
                    if nd:
                        hinge(dsq_dve, nd, 1, "d")

                # taper chunks: all-DVE so the ACT table stays on Sqrt
                r0 = n_main * chunk_a
                tcol = 0
                for i, a in enumerate(taper):
                    rows = slice(r0 * P, (r0 + a) * P)
                    tfd = a * D
                    xt = tio.tile([P, tfd], F32, tag=f"txt{i}")
                    yt = tio.tile([P, tfd], F32, tag=f"tyt{i}")
                    zt = tio.tile([P, tfd], F32, tag=f"tzt{i}")
                    if mode != "compute":
                        load(xt, yt, zt, rows)
                    r0 += a
                    if mode == "dma":
                        continue
                    nc.vector.tensor_sub(yt[:], xt[:], yt[:])
                    nc.vector.tensor_sub(zt[:], xt[:], zt[:])
                    # tensor_tensor_reduce would fuse square+rowsum, but it
                    # aborts on HW via this runtime (sim-only); use mult +
                    # reduce_sum instead.
                    for half, t in ((0, yt), (1, zt)):
                        col = half * nt + tcol
                        nc.vector.tensor_mul(t[:], t[:], t[:])
                        nc.vector.reduce_sum(
                            dsq_tap[:, col : col + a],
                            t[:].rearrange("p (a d) -> p a d", a=a),
                            axis=mybir.AxisListType.X,
                        )
                    tcol += a

                if mode != "full":
                    return
                # tail B (the drain): ACT table is already Sqrt
                hinge(dsq_tap, nt, 2, "t")
                nc.sync.dma_start(out[:], hsum[:])

            if loop and repeat > 1:
                with tc.For_i(0, repeat, 1):
                    rep_body()
            else:
                for _ in range(repeat):
                    rep_body()
    nc.compile()
    return nc


def _run(nc: bass.Bass, x, y, z):
    in_maps = [
        {
            "x": np.ascontiguousarray(x[i * SHARD : (i + 1) * SHARD]),
            "y": np.ascontiguousarray(y[i * SHARD : (i + 1) * SHARD]),
            "z": np.ascontiguousarray(z[i * SHARD : (i + 1) * SHARD]),
        }
        for i in range(N_CORES)
    ]
    return bass_utils.run_bass_kernel_spmd(
        nc, in_maps, core_ids=list(range(N_CORES))
    )


_NC_CACHE = None


def kernel(x: np.ndarray, y: np.ndarray, z: np.ndarray) -> np.ndarray:
    global _NC_CACHE
    x = np.asarray(x, dtype=np.float32)
    y = np.asarray(y, dtype=np.float32)
    z = np.asarray(z, dtype=np.float32)
    if _NC_CACHE is None:
        _NC_CACHE = build_nc(1)
    res = _run(_NC_CACHE, x, y, z)
    total = np.float64(0.0)
    for r in res.results:
        total += r["out"].astype(np.float64).sum()
    return np.float32(total)
